# revision 25
# baseline (speedup 1.0000x reference)
"""Gemma3 sliding-window attention layer on 8 Trainium2 NeuronCores.

Sharding: tensor-parallel over heads. Core c computes q-head c and kv-head c//2
(kv heads are duplicated across the 2 cores sharing them), then the o_proj
row-slice for its head. The 8 partial o_proj outputs are summed on the host
(the unshard step for row-sharded o_proj).

Device kernel layout choices (v2, bf16 + 512-token blocks):
- all matmul operands are bf16 (fp32 PSUM accumulation): halves LDWEIGHTS
  time and SBUF/DMA traffic; fp32r already streams at 1 cyc/row so the
  moving-dim rate is unchanged.
- 512-token blocks double the moving dim per weight load vs 256.
- q/k are produced in [d, tok] layout (weights stationary); v in [tok, d]
  (hidden stationary). scoresT [keys, q] keeps softmax sums and the PV
  contraction on the partition (keys) axis.
- rmsnorm rstd and softmax 1/sum chains avoid the slow DVE reciprocal
  (reciprocal_approx_fast) and the PE broadcast matmuls (gpsimd
  partition_broadcast), keeping the PE queue free of serialized scalar work.
- attention inner loop is software-pipelined: scores(kt+1) issue before
  sums/pv(kt) so exp/mask of kt overlaps PE work.
- o_proj for block t runs at the start of block t+1, covering the k-rope and
  inv-sum chains with PE work.
"""
import os
import sys
import types
import contextlib
import ctypes

import numpy as np
import ml_dtypes

for _p in ("/opt/trn_rl_repo", "/root/.axon_site/_ro/trn_rl_repo"):
    if os.path.isdir(_p) and _p not in sys.path:
        sys.path.insert(0, _p)

from contextlib import ExitStack

import concourse.bass as bass
import concourse.mybir as mybir
import concourse.tile as tile
from concourse import bacc
from concourse.bass_utils import run_bass_kernel_spmd

S = 4096
HID = 2560
NH = 8
NKV = 4
HD = 256
WIN = 1024
ROPE_BASE = 10000.0
EPS = 1e-6
SCALING = HD ** -0.5

NCORES = 8
BLK = 512           # tokens per block
NBLK = S // BLK     # 8
KT = HID // 128     # 20 hid k-tiles
f32 = mybir.dt.float32
bf16 = mybir.dt.bfloat16
AF = mybir.ActivationFunctionType
bfnp = ml_dtypes.bfloat16

_NC = None
_last_results = None


def _install_ntff_shim():
    """antenv.axon_hooks is absent in this image; rebuild it over libaxon so
    run_bass_kernel_spmd(trace=True) can capture NTFF profiles."""
    if "antenv.axon_hooks" in sys.modules:
        return
    so_path = "/opt/axon/libaxon_pjrt.so"
    hook = None
    try:
        lib = ctypes.CDLL(so_path)
        if hasattr(lib, "axon_start_nrt_profile"):
            lib.axon_start_nrt_profile.argtypes = [
                ctypes.POINTER(ctypes.c_int64),
                ctypes.c_size_t,
            ]
            lib.axon_start_nrt_profile.restype = ctypes.c_int64
            lib.axon_stop_nrt_profile.argtypes = [ctypes.c_char_p]
            lib.axon_stop_nrt_profile.restype = ctypes.c_int64

            @contextlib.contextmanager
            def _hook(output_dir, device_ids):
                import jax

                jax.devices()
                if device_ids:
                    ids = (ctypes.c_int64 * len(device_ids))(*device_ids)
                    rc = lib.axon_start_nrt_profile(ids, len(device_ids))
                else:
                    rc = lib.axon_start_nrt_profile(None, 0)
                if rc != 0:
                    raise RuntimeError(f"axon_start_nrt_profile rc={rc}")
                try:
                    yield
                finally:
                    n = lib.axon_stop_nrt_profile(str(output_dir).encode())
                    if n < 0:
                        raise RuntimeError(f"axon_stop_nrt_profile rc={n}")

            hook = _hook
    except OSError:
        pass
    mod = types.ModuleType("antenv.axon_hooks")
    mod.get_axon_ntff_profile_hook = lambda: hook
    mod.set_axon_ntff_profile_hook = lambda h: None
    sys.modules["antenv.axon_hooks"] = mod


def _body(ctx, tc, hT, w, ow, cs, msk, nw, on, outp):
    nc = tc.nc

    const = ctx.enter_context(tc.tile_pool(name="const", bufs=1))
    hpool = ctx.enter_context(tc.tile_pool(name="hT", bufs=2))
    cspool = ctx.enter_context(tc.tile_pool(name="cs", bufs=2))
    qpool = ctx.enter_context(tc.tile_pool(name="qT", bufs=2))
    kpool = ctx.enter_context(tc.tile_pool(name="kT", bufs=4))
    vpool = ctx.enter_context(tc.tile_pool(name="v", bufs=8))
    sqpool = ctx.enter_context(tc.tile_pool(name="sq", bufs=2))
    xpool = ctx.enter_context(tc.tile_pool(name="x", bufs=4))
    mpool = ctx.enter_context(tc.tile_pool(name="m", bufs=6))
    smsb = ctx.enter_context(tc.tile_pool(name="smsb", bufs=4))
    rbpool = ctx.enter_context(tc.tile_pool(name="rb", bufs=3))
    prpool = ctx.enter_context(tc.tile_pool(name="pr", bufs=3))
    atpool = ctx.enter_context(tc.tile_pool(name="at", bufs=4))
    obpool = ctx.enter_context(tc.tile_pool(name="ob", bufs=2))

    qkp = ctx.enter_context(tc.tile_pool(name="qkp", bufs=2, space="PSUM"))
    vop = ctx.enter_context(tc.tile_pool(name="vop", bufs=1, space="PSUM"))
    bigp = ctx.enter_context(tc.tile_pool(name="bigp", bufs=2, space="PSUM"))
    pvp = ctx.enter_context(tc.tile_pool(name="pvp", bufs=2, space="PSUM"))
    smp = ctx.enter_context(tc.tile_pool(name="smp", bufs=1, space="PSUM"))

    # resident constants; w and block-0 hT interleaved per k-tile so the
    # first projection matmuls start as soon as their slices land
    w_sb = const.tile([128, KT * 768], bf16)
    ow_sb = const.tile([128, 2 * HID], bf16)
    msk_sb = const.tile([128, 256], bf16)       # [edge | diag]
    nc.sync.dma_start(out=msk_sb, in_=msk)
    nw_sb = const.tile([128, 4], f32)
    nc.sync.dma_start(out=nw_sb, in_=nw)
    ones_sb = const.tile([128, 128], bf16)
    nc.sync.dma_start(out=ones_sb, in_=on)

    h_tiles = {}
    cs_tiles = {}
    k_tiles = {}
    v_tiles = {}
    at_tiles = {}

    def prefetch(t):
        if t >= NBLK:
            return
        hTt = hpool.tile([128, KT * BLK], bf16, tag="hT")
        if t == 0:
            for k in range(KT):
                nc.sync.dma_start(out=w_sb[:, k * 768:(k + 1) * 768],
                                  in_=w[:, k * 768:(k + 1) * 768])
                nc.sync.dma_start(
                    out=hTt[:, k * BLK:(k + 1) * BLK],
                    in_=hT[:, k * BLK:(k + 1) * BLK])
            nc.sync.dma_start(out=ow_sb, in_=ow)
        else:
            nc.sync.dma_start(out=hTt,
                              in_=hT[:, t * KT * BLK:(t + 1) * KT * BLK])
        h_tiles[t] = hTt
        cst = cspool.tile([128, 2 * BLK], bf16, tag="cs")
        nc.sync.dma_start(out=cst, in_=cs[:, t * 2 * BLK:(t + 1) * 2 * BLK])
        cs_tiles[t] = cst

    def qk_head(t, j0, wo, dest):
        """projection -> rmsnorm -> rope for q (j0=0) or k (j0=2), [d, tok]."""
        hTt = h_tiles[t]
        cst = cs_tiles[t]
        cos = cst[:, 0:BLK]
        sin = cst[:, BLK:2 * BLK]
        xps = []
        for j in (j0, j0 + 1):
            ps = qkp.tile([128, BLK], f32, tag="qk")
            for k in range(KT):
                nc.tensor.matmul(
                    ps,
                    w_sb[:, k * 768 + j * 128:k * 768 + (j + 1) * 128],
                    hTt[:, k * BLK:(k + 1) * BLK],
                    start=(k == 0), stop=(k == KT - 1))
            xps.append(ps)
        x0p, x1p = xps
        # sum of squares over head_dim via all-ones matmul: every output
        # partition receives the full sum (the broadcast comes for free)
        sq0 = sqpool.tile([128, BLK], bf16, tag="sq")
        sq1 = sqpool.tile([128, BLK], bf16, tag="sq")
        nc.scalar.activation(sq0, x0p, AF.Square)
        nc.scalar.activation(sq1, x1p, AF.Square)
        ssqb = smp.tile([128, BLK], f32, tag="sm")
        nc.tensor.matmul(ssqb, ones_sb, sq0, start=True, stop=False)
        nc.tensor.matmul(ssqb, ones_sb, sq1, start=False, stop=True)
        # evacuate x*(1+w) from psum on ACT
        x0 = xpool.tile([128, BLK], bf16, tag="x")
        x1 = xpool.tile([128, BLK], bf16, tag="x")
        nc.scalar.activation(x0, x0p, AF.Copy, bias=0.0,
                             scale=nw_sb[:, wo:wo + 1])
        nc.scalar.activation(x1, x1p, AF.Copy, bias=0.0,
                             scale=nw_sb[:, wo + 1:wo + 2])
        # rstd = sqrt(1/(mean+eps)), ending in bf16 so rope muls stay 2x
        t1 = smsb.tile([128, BLK], f32, tag="s1")
        nc.vector.tensor_scalar(t1, ssqb, 1.0 / HD, EPS,
                                mybir.AluOpType.mult, mybir.AluOpType.add)
        t2 = smsb.tile([128, BLK], f32, tag="s1")
        nc.vector.reciprocal_approx_fast(out=t2, in_=t1)
        rb = rbpool.tile([128, BLK], bf16, tag="rb")
        nc.scalar.activation(rb, t2, AF.Sqrt)
        # rope mix
        a = mpool.tile([128, BLK], bf16, tag="m")
        nc.vector.tensor_mul(a, x0, cos)
        b = mpool.tile([128, BLK], bf16, tag="m")
        nc.vector.tensor_mul(b, x1, sin)
        e = mpool.tile([128, BLK], bf16, tag="m")
        nc.vector.tensor_sub(e, a, b)
        c_ = mpool.tile([128, BLK], bf16, tag="m")
        nc.vector.tensor_mul(c_, x1, cos)
        d = mpool.tile([128, BLK], bf16, tag="m")
        nc.vector.tensor_mul(d, x0, sin)
        f = mpool.tile([128, BLK], bf16, tag="m")
        nc.vector.tensor_add(f, c_, d)
        nc.vector.tensor_mul(dest[:, 0:BLK], e, rb)
        nc.vector.tensor_mul(dest[:, BLK:2 * BLK], f, rb)

    def v_proj(t):
        hTt = h_tiles[t]
        for half in range(2):
            vps = vop.tile([128, 2 * HD], f32, tag="vps")
            for st2 in range(2):
                st = half * 2 + st2
                dst = vps[:, st2 * HD:(st2 + 1) * HD]
                for k in range(KT):
                    nc.tensor.matmul(
                        dst,
                        hTt[:, k * BLK + st * 128:k * BLK + st * 128 + 128],
                        w_sb[:, k * 768 + 512:(k + 1) * 768],
                        start=(k == 0), stop=(k == KT - 1))
            vt = vpool.tile([128, 2 * HD], bf16, tag="v")
            nc.scalar.activation(vt, vps, AF.Copy, bias=0.0, scale=1.0)
            v_tiles[4 * t + half * 2] = (vt, 0)
            v_tiles[4 * t + half * 2 + 1] = (vt, HD)

    def attention(t):
        qTt = at_tiles.pop(("q", t))
        pv0 = pvp.tile([128, BLK], f32, tag="pv")
        pv1 = pvp.tile([128, BLK], f32, tag="pv")
        sums = smp.tile([128, BLK], f32, tag="sm")
        kts = list(range(max(0, 4 * t - 8), 4 * t + 4))
        last_i = len(kts) - 1

        def emit_pv(kt, pr, i):
            first, last = (i == 0), (i == last_i)
            vt, vo = v_tiles[kt]
            nc.tensor.matmul(sums, ones_sb, pr, start=first, stop=last)
            nc.tensor.matmul(pv0, vt[:, vo:vo + 128], pr,
                             start=first, stop=last)
            nc.tensor.matmul(pv1, vt[:, vo + 128:vo + 256], pr,
                             start=first, stop=last)

        prev = None
        for i, kt in enumerate(kts):
            ct, sb = kt // 4, kt % 4
            ksrc = k_tiles[ct]
            # valid q-subtiles for this key tile form a contiguous range;
            # only compute scores/exp there, memset-zero the rest of pr
            dt_ = kt - 4 * t
            qlo = max(0, dt_) * 128
            qhi = min(4, dt_ + 9) * 128
            sc = bigp.tile([128, BLK], f32, tag="big")
            nc.tensor.matmul(sc[:, qlo:qhi], ksrc[:, sb * 128:sb * 128 + 128],
                             qTt[:, qlo:qhi], start=True, stop=False)
            nc.tensor.matmul(sc[:, qlo:qhi],
                             ksrc[:, BLK + sb * 128:BLK + sb * 128 + 128],
                             qTt[:, BLK + qlo:BLK + qhi],
                             start=False, stop=True)
            if prev is not None:
                emit_pv(*prev)
            pr = prpool.tile([128, BLK], bf16, tag="pr")
            nc.scalar.activation(pr[:, qlo:qhi], sc[:, qlo:qhi], AF.Exp,
                                 bias=0.0, scale=SCALING)
            if qlo > 0:
                nc.gpsimd.memset(pr[:, 0:qlo], 0.0)
            if qhi < BLK:
                nc.gpsimd.memset(pr[:, qhi:BLK], 0.0)
            if 0 <= dt_ <= 3:
                sl = slice(dt_ * 128, (dt_ + 1) * 128)
                nc.vector.tensor_mul(pr[:, sl], pr[:, sl], msk_sb[:, 128:256])
            if -8 <= dt_ <= -5:
                s = dt_ + 8
                sl = slice(s * 128, (s + 1) * 128)
                nc.vector.tensor_mul(pr[:, sl], pr[:, sl], msk_sb[:, 0:128])
            prev = (kt, pr, i)
        emit_pv(*prev)

        # 1/sums (already broadcast across partitions) -> normalize pv
        ib = smsb.tile([128, BLK], f32, tag="s1")
        nc.vector.reciprocal_approx_fast(out=ib, in_=sums)
        at0 = atpool.tile([128, BLK], bf16, tag="at")
        at1 = atpool.tile([128, BLK], bf16, tag="at")
        nc.vector.tensor_mul(at0, pv0, ib)
        nc.vector.tensor_mul(at1, pv1, ib)
        at_tiles[t] = (at0, at1)

    def o_proj(u):
        at0, at1 = at_tiles.pop(u)
        for st in range(4):
            ob = obpool.tile([128, HID], bf16, tag="ob")
            for hc in range(HID // BLK):
                op = bigp.tile([128, BLK], f32, tag="big")
                nc.tensor.matmul(op, at0[:, st * 128:(st + 1) * 128],
                                 ow_sb[:, hc * BLK:(hc + 1) * BLK],
                                 start=True, stop=False)
                nc.tensor.matmul(op, at1[:, st * 128:(st + 1) * 128],
                                 ow_sb[:, HID + hc * BLK:HID + (hc + 1) * BLK],
                                 start=False, stop=True)
                eng = (st * 5 + hc) % 2
                dst = ob[:, hc * BLK:(hc + 1) * BLK]
                if eng == 0:
                    nc.scalar.activation(dst, op, AF.Copy, bias=0.0, scale=1.0)
                else:
                    nc.vector.tensor_copy(dst, op)
                nc.sync.dma_start(
                    out=outp[u * BLK + st * 128:u * BLK + (st + 1) * 128,
                             hc * BLK:(hc + 1) * BLK],
                    in_=dst)

    prefetch(0)
    for t in range(NBLK):
        prefetch(t + 1)
        qTt = qpool.tile([128, 2 * BLK], bf16, tag="qT")
        qk_head(t, 0, 0, qTt)
        at_tiles[("q", t)] = qTt
        v_proj(t)
        kTt = kpool.tile([128, 2 * BLK], bf16, tag="kT")
        qk_head(t, 2, 2, kTt)
        k_tiles[t] = kTt
        h_tiles.pop(t)
        cs_tiles.pop(t)
        if t > 0:
            o_proj(t - 1)
        attention(t)
    o_proj(NBLK - 1)


def _build():
    nc = bacc.Bacc("TRN2", target_bir_lowering=False, debug=False,
                   num_devices=NCORES)
    hT = nc.dram_tensor("hT", [128, KT * S], bf16, kind="ExternalInput").ap()
    w = nc.dram_tensor("w", [128, KT * 768], bf16, kind="ExternalInput").ap()
    ow = nc.dram_tensor("ow", [128, 2 * HID], bf16, kind="ExternalInput").ap()
    cs = nc.dram_tensor("cs", [128, NBLK * 2 * BLK], bf16,
                        kind="ExternalInput").ap()
    msk = nc.dram_tensor("msk", [128, 256], bf16, kind="ExternalInput").ap()
    nw = nc.dram_tensor("nw", [128, 4], f32, kind="ExternalInput").ap()
    on = nc.dram_tensor("on", [128, 128], bf16, kind="ExternalInput").ap()
    outp = nc.dram_tensor("outp", [S, HID], bf16, kind="ExternalOutput").ap()
    with tile.TileContext(nc) as tc, ExitStack() as ctx:
        with nc.allow_low_precision(reason="bf16 matmul pipeline"):
            _body(ctx, tc, hT, w, ow, cs, msk, nw, on, outp)
    nc.compile()
    return nc


def _get_nc():
    global _NC
    if _NC is None:
        _NC = _build()
    return _NC


def kernel(positions, hidden_states, qkv_w, o_w, q_norm_w, k_norm_w):
    global _last_results
    _install_ntff_shim()

    positions = np.asarray(positions)
    hidden_states = np.asarray(hidden_states, dtype=np.float32)
    qkv_w = np.asarray(qkv_w, dtype=np.float32)
    o_w = np.asarray(o_w, dtype=np.float32)
    q_norm_w = np.asarray(q_norm_w, dtype=np.float32)
    k_norm_w = np.asarray(k_norm_w, dtype=np.float32)
    assert np.array_equal(positions.astype(np.int64), np.arange(S)), \
        "kernel assumes contiguous arange positions (banded sliding window)"

    hT0 = hidden_states.T  # [HID, S]
    hT = np.ascontiguousarray(
        hT0.reshape(KT, 128, NBLK, BLK).transpose(1, 2, 0, 3)
        .reshape(128, KT * S)).astype(bfnp)

    inv_freq = 1.0 / (ROPE_BASE ** (np.arange(0, HD, 2, dtype=np.float32) / HD))
    freqs = positions.astype(np.float32)[:, None] * inv_freq[None, :]  # [S,128]
    cos_t = np.cos(freqs).T.astype(np.float32)
    sin_t = np.sin(freqs).T.astype(np.float32)
    csb = np.stack([cos_t.reshape(128, NBLK, BLK),
                    sin_t.reshape(128, NBLK, BLK)], axis=2)
    cs = np.ascontiguousarray(
        csb.reshape(128, NBLK * 2 * BLK)).astype(bfnp)

    kl = np.arange(128)[:, None]
    ql = np.arange(128)[None, :]
    edge = (kl > ql).astype(np.float32)
    diag = (kl <= ql).astype(np.float32)
    msk = np.concatenate([edge, diag], axis=1).astype(bfnp)  # [128, 256]

    nwq = 1.0 + q_norm_w
    nwk = 1.0 + k_norm_w
    nw = np.stack([nwq[:128], nwq[128:], nwk[:128], nwk[128:]], axis=1)
    nw = np.ascontiguousarray(nw.astype(np.float32))  # [128, 4]

    on = np.ones((128, 128), bfnp)

    in_maps = []
    for c in range(NCORES):
        g = c // 2
        wq = qkv_w[:, c * HD:(c + 1) * HD]
        wk = qkv_w[:, NH * HD + g * HD:NH * HD + (g + 1) * HD]
        wv = qkv_w[:, (NH + NKV) * HD + g * HD:(NH + NKV) * HD + (g + 1) * HD]
        wslice = np.concatenate([wq, wk, wv], axis=1).astype(np.float32)
        wslice = np.ascontiguousarray(
            wslice.reshape(KT, 128, 768).transpose(1, 0, 2)
            .reshape(128, KT * 768)).astype(bfnp)
        owslice = o_w[c * HD:(c + 1) * HD, :].astype(np.float32)
        owslice = np.ascontiguousarray(
            owslice.reshape(2, 128, HID).transpose(1, 0, 2)
            .reshape(128, 2 * HID)).astype(bfnp)
        in_maps.append({
            "hT": hT, "w": wslice, "ow": owslice, "cs": cs, "msk": msk,
            "nw": nw, "on": on,
        })

    nc = _get_nc()
    res = run_bass_kernel_spmd(nc, in_maps, list(range(NCORES)))
    _last_results = res

    out = res.results[0]["outp"].astype(np.float32)
    for c in range(1, NCORES):
        out += res.results[c]["outp"].astype(np.float32)
    return out


# revision 26
# speedup vs baseline: 1.1897x; 1.1897x over previous
"""Gemma3 sliding-window attention layer on 8 Trainium2 NeuronCores.

Sharding: tensor-parallel over heads. Core c computes q-head c and kv-head c//2
(kv heads are duplicated across the 2 cores sharing them), then the o_proj
row-slice for its head. The 8 partial o_proj outputs are summed on the host
(the unshard step for row-sharded o_proj).

Device kernel layout choices (v2, bf16 + 512-token blocks):
- all matmul operands are bf16 (fp32 PSUM accumulation): halves LDWEIGHTS
  time and SBUF/DMA traffic; fp32r already streams at 1 cyc/row so the
  moving-dim rate is unchanged.
- 512-token blocks double the moving dim per weight load vs 256.
- q/k are produced in [d, tok] layout (weights stationary); v in [tok, d]
  (hidden stationary). scoresT [keys, q] keeps softmax sums and the PV
  contraction on the partition (keys) axis.
- rmsnorm rstd and softmax 1/sum chains avoid the slow DVE reciprocal
  (reciprocal_approx_fast) and the PE broadcast matmuls (gpsimd
  partition_broadcast), keeping the PE queue free of serialized scalar work.
- attention inner loop is software-pipelined: scores(kt+1) issue before
  sums/pv(kt) so exp/mask of kt overlaps PE work.
- o_proj for block t runs at the start of block t+1, covering the k-rope and
  inv-sum chains with PE work.
"""
import os
import sys
import types
import contextlib
import ctypes

import numpy as np
import ml_dtypes

for _p in ("/opt/trn_rl_repo", "/root/.axon_site/_ro/trn_rl_repo"):
    if os.path.isdir(_p) and _p not in sys.path:
        sys.path.insert(0, _p)

from contextlib import ExitStack

import concourse.bass as bass
import concourse.mybir as mybir
import concourse.tile as tile
from concourse import bacc
from concourse.bass_utils import run_bass_kernel_spmd

S = 4096
HID = 2560
NH = 8
NKV = 4
HD = 256
WIN = 1024
ROPE_BASE = 10000.0
EPS = 1e-6
SCALING = HD ** -0.5

NCORES = 8
BLK = 512           # tokens per block
NBLK = S // BLK     # 8
KT = HID // 128     # 20 hid k-tiles
f32 = mybir.dt.float32
bf16 = mybir.dt.bfloat16
AF = mybir.ActivationFunctionType
bfnp = ml_dtypes.bfloat16

_NC = None
_last_results = None


def _install_ntff_shim():
    """antenv.axon_hooks is absent in this image; rebuild it over libaxon so
    run_bass_kernel_spmd(trace=True) can capture NTFF profiles."""
    if "antenv.axon_hooks" in sys.modules:
        return
    so_path = "/opt/axon/libaxon_pjrt.so"
    hook = None
    try:
        lib = ctypes.CDLL(so_path)
        if hasattr(lib, "axon_start_nrt_profile"):
            lib.axon_start_nrt_profile.argtypes = [
                ctypes.POINTER(ctypes.c_int64),
                ctypes.c_size_t,
            ]
            lib.axon_start_nrt_profile.restype = ctypes.c_int64
            lib.axon_stop_nrt_profile.argtypes = [ctypes.c_char_p]
            lib.axon_stop_nrt_profile.restype = ctypes.c_int64

            @contextlib.contextmanager
            def _hook(output_dir, device_ids):
                import jax

                jax.devices()
                if device_ids:
                    ids = (ctypes.c_int64 * len(device_ids))(*device_ids)
                    rc = lib.axon_start_nrt_profile(ids, len(device_ids))
                else:
                    rc = lib.axon_start_nrt_profile(None, 0)
                if rc != 0:
                    raise RuntimeError(f"axon_start_nrt_profile rc={rc}")
                try:
                    yield
                finally:
                    n = lib.axon_stop_nrt_profile(str(output_dir).encode())
                    if n < 0:
                        raise RuntimeError(f"axon_stop_nrt_profile rc={n}")

            hook = _hook
    except OSError:
        pass
    mod = types.ModuleType("antenv.axon_hooks")
    mod.get_axon_ntff_profile_hook = lambda: hook
    mod.set_axon_ntff_profile_hook = lambda h: None
    sys.modules["antenv.axon_hooks"] = mod


def _body(ctx, tc, hT, w, ow, cs, msk, nw, on, outp):
    nc = tc.nc

    const = ctx.enter_context(tc.tile_pool(name="const", bufs=1))
    hpool = ctx.enter_context(tc.tile_pool(name="hT", bufs=2))
    cspool = ctx.enter_context(tc.tile_pool(name="cs", bufs=2))
    qpool = ctx.enter_context(tc.tile_pool(name="qT", bufs=2))
    kpool = ctx.enter_context(tc.tile_pool(name="kT", bufs=4))
    vpool = ctx.enter_context(tc.tile_pool(name="v", bufs=8))
    sqpool = ctx.enter_context(tc.tile_pool(name="sq", bufs=2))
    xpool = ctx.enter_context(tc.tile_pool(name="x", bufs=4))
    mpool = ctx.enter_context(tc.tile_pool(name="m", bufs=6))
    smsb = ctx.enter_context(tc.tile_pool(name="smsb", bufs=4))
    rbpool = ctx.enter_context(tc.tile_pool(name="rb", bufs=3))
    prpool = ctx.enter_context(tc.tile_pool(name="pr", bufs=3))
    atpool = ctx.enter_context(tc.tile_pool(name="at", bufs=4))
    obpool = ctx.enter_context(tc.tile_pool(name="ob", bufs=2))

    qkp = ctx.enter_context(tc.tile_pool(name="qkp", bufs=2, space="PSUM"))
    vop = ctx.enter_context(tc.tile_pool(name="vop", bufs=1, space="PSUM"))
    bigp = ctx.enter_context(tc.tile_pool(name="bigp", bufs=2, space="PSUM"))
    pvp = ctx.enter_context(tc.tile_pool(name="pvp", bufs=2, space="PSUM"))
    smp = ctx.enter_context(tc.tile_pool(name="smp", bufs=1, space="PSUM"))

    # resident constants; w and block-0 hT interleaved per k-tile so the
    # first projection matmuls start as soon as their slices land
    w_sb = const.tile([128, KT * 768], bf16)
    ow_sb = const.tile([128, 2 * HID], bf16)
    msk_sb = const.tile([128, 256], bf16)       # [edge | diag]
    nc.sync.dma_start(out=msk_sb, in_=msk)
    nw_sb = const.tile([128, 4], f32)
    nc.sync.dma_start(out=nw_sb, in_=nw)
    ones_sb = const.tile([128, 128], bf16)
    nc.sync.dma_start(out=ones_sb, in_=on)

    h_tiles = {}
    cs_tiles = {}
    k_tiles = {}
    v_tiles = {}
    at_tiles = {}

    def prefetch(t):
        if t >= NBLK:
            return
        hTt = hpool.tile([128, KT * BLK], bf16, tag="hT")
        if t == 0:
            for k in range(KT):
                nc.sync.dma_start(out=w_sb[:, k * 768:(k + 1) * 768],
                                  in_=w[:, k * 768:(k + 1) * 768])
                nc.sync.dma_start(
                    out=hTt[:, k * BLK:(k + 1) * BLK],
                    in_=hT[:, k * BLK:(k + 1) * BLK])
            nc.sync.dma_start(out=ow_sb, in_=ow)
        else:
            nc.sync.dma_start(out=hTt,
                              in_=hT[:, t * KT * BLK:(t + 1) * KT * BLK])
        h_tiles[t] = hTt
        cst = cspool.tile([128, 2 * BLK], bf16, tag="cs")
        nc.sync.dma_start(out=cst, in_=cs[:, t * 2 * BLK:(t + 1) * 2 * BLK])
        cs_tiles[t] = cst

    def qk_head(t, j0, wo, dest):
        """projection -> rmsnorm -> rope for q (j0=0) or k (j0=2), [d, tok]."""
        hTt = h_tiles[t]
        cst = cs_tiles[t]
        cos = cst[:, 0:BLK]
        sin = cst[:, BLK:2 * BLK]
        xps = []
        for j in (j0, j0 + 1):
            ps = qkp.tile([128, BLK], f32, tag="qk")
            for k in range(KT):
                nc.tensor.matmul(
                    ps,
                    w_sb[:, k * 768 + j * 128:k * 768 + (j + 1) * 128],
                    hTt[:, k * BLK:(k + 1) * BLK],
                    start=(k == 0), stop=(k == KT - 1))
            xps.append(ps)
        x0p, x1p = xps
        # sum of squares over head_dim via all-ones matmul: every output
        # partition receives the full sum (the broadcast comes for free)
        sq0 = sqpool.tile([128, BLK], bf16, tag="sq")
        sq1 = sqpool.tile([128, BLK], bf16, tag="sq")
        nc.scalar.activation(sq0, x0p, AF.Square)
        nc.scalar.activation(sq1, x1p, AF.Square)
        ssqb = smp.tile([128, BLK], f32, tag="sm")
        nc.tensor.matmul(ssqb, ones_sb, sq0, start=True, stop=False)
        nc.tensor.matmul(ssqb, ones_sb, sq1, start=False, stop=True)
        # evacuate x*(1+w) from psum on ACT
        x0 = xpool.tile([128, BLK], bf16, tag="x")
        x1 = xpool.tile([128, BLK], bf16, tag="x")
        nc.scalar.activation(x0, x0p, AF.Copy, bias=0.0,
                             scale=nw_sb[:, wo:wo + 1])
        nc.scalar.activation(x1, x1p, AF.Copy, bias=0.0,
                             scale=nw_sb[:, wo + 1:wo + 2])
        # rstd = sqrt(1/(mean+eps)), ending in bf16 so rope muls stay 2x
        t1 = smsb.tile([128, BLK], f32, tag="s1")
        nc.vector.tensor_scalar(t1, ssqb, 1.0 / HD, EPS,
                                mybir.AluOpType.mult, mybir.AluOpType.add)
        t2 = smsb.tile([128, BLK], f32, tag="s1")
        nc.vector.reciprocal_approx_fast(out=t2, in_=t1)
        rb = rbpool.tile([128, BLK], bf16, tag="rb")
        nc.scalar.activation(rb, t2, AF.Sqrt)
        # rope mix
        a = mpool.tile([128, BLK], bf16, tag="m")
        nc.vector.tensor_mul(a, x0, cos)
        b = mpool.tile([128, BLK], bf16, tag="m")
        nc.vector.tensor_mul(b, x1, sin)
        e = mpool.tile([128, BLK], bf16, tag="m")
        nc.vector.tensor_sub(e, a, b)
        c_ = mpool.tile([128, BLK], bf16, tag="m")
        nc.vector.tensor_mul(c_, x1, cos)
        d = mpool.tile([128, BLK], bf16, tag="m")
        nc.vector.tensor_mul(d, x0, sin)
        f = mpool.tile([128, BLK], bf16, tag="m")
        nc.vector.tensor_add(f, c_, d)
        nc.vector.tensor_mul(dest[:, 0:BLK], e, rb)
        nc.vector.tensor_mul(dest[:, BLK:2 * BLK], f, rb)

    def v_proj(t):
        hTt = h_tiles[t]
        for half in range(2):
            vps = vop.tile([128, 2 * HD], f32, tag="vps")
            for st2 in range(2):
                st = half * 2 + st2
                dst = vps[:, st2 * HD:(st2 + 1) * HD]
                for k in range(KT):
                    nc.tensor.matmul(
                        dst,
                        hTt[:, k * BLK + st * 128:k * BLK + st * 128 + 128],
                        w_sb[:, k * 768 + 512:(k + 1) * 768],
                        start=(k == 0), stop=(k == KT - 1))
            vt = vpool.tile([128, 2 * HD], bf16, tag="v")
            nc.scalar.activation(vt, vps, AF.Copy, bias=0.0, scale=1.0)
            v_tiles[4 * t + half * 2] = (vt, 0)
            v_tiles[4 * t + half * 2 + 1] = (vt, HD)

    def attention(t):
        qTt = at_tiles.pop(("q", t))
        pv0 = pvp.tile([128, BLK], f32, tag="pv")
        pv1 = pvp.tile([128, BLK], f32, tag="pv")
        sums = smp.tile([128, BLK], f32, tag="sm")
        kts = list(range(max(0, 4 * t - 8), 4 * t + 4))
        last_i = len(kts) - 1

        def emit_pv(kt, pr, i):
            first, last = (i == 0), (i == last_i)
            vt, vo = v_tiles[kt]
            nc.tensor.matmul(sums, ones_sb, pr, start=first, stop=last)
            nc.tensor.matmul(pv0, vt[:, vo:vo + 128], pr,
                             start=first, stop=last)
            nc.tensor.matmul(pv1, vt[:, vo + 128:vo + 256], pr,
                             start=first, stop=last)

        prev = None
        for i, kt in enumerate(kts):
            ct, sb = kt // 4, kt % 4
            ksrc = k_tiles[ct]
            # valid q-subtiles for this key tile form a contiguous range;
            # only compute scores/exp there, memset-zero the rest of pr
            dt_ = kt - 4 * t
            qlo = max(0, dt_) * 128
            qhi = min(4, dt_ + 9) * 128
            sc = bigp.tile([128, BLK], f32, tag="big")
            nc.tensor.matmul(sc[:, qlo:qhi], ksrc[:, sb * 128:sb * 128 + 128],
                             qTt[:, qlo:qhi], start=True, stop=False)
            nc.tensor.matmul(sc[:, qlo:qhi],
                             ksrc[:, BLK + sb * 128:BLK + sb * 128 + 128],
                             qTt[:, BLK + qlo:BLK + qhi],
                             start=False, stop=True)
            if prev is not None:
                emit_pv(*prev)
            pr = prpool.tile([128, BLK], bf16, tag="pr")
            nc.scalar.activation(pr[:, qlo:qhi], sc[:, qlo:qhi], AF.Exp,
                                 bias=0.0, scale=SCALING)
            if qlo > 0:
                nc.gpsimd.memset(pr[:, 0:qlo], 0.0)
            if qhi < BLK:
                nc.gpsimd.memset(pr[:, qhi:BLK], 0.0)
            if 0 <= dt_ <= 3:
                sl = slice(dt_ * 128, (dt_ + 1) * 128)
                nc.vector.tensor_mul(pr[:, sl], pr[:, sl], msk_sb[:, 128:256])
            if -8 <= dt_ <= -5:
                s = dt_ + 8
                sl = slice(s * 128, (s + 1) * 128)
                nc.vector.tensor_mul(pr[:, sl], pr[:, sl], msk_sb[:, 0:128])
            prev = (kt, pr, i)
        emit_pv(*prev)

        # 1/sums (already broadcast across partitions) -> normalize pv
        ib = smsb.tile([128, BLK], f32, tag="s1")
        nc.vector.reciprocal_approx_fast(out=ib, in_=sums)
        at0 = atpool.tile([128, BLK], bf16, tag="at")
        at1 = atpool.tile([128, BLK], bf16, tag="at")
        nc.vector.tensor_mul(at0, pv0, ib)
        nc.vector.tensor_mul(at1, pv1, ib)
        at_tiles[t] = (at0, at1)

    def o_proj(u):
        at0, at1 = at_tiles.pop(u)
        for st in range(4):
            ob = obpool.tile([128, HID], bf16, tag="ob")
            for hc in range(HID // BLK):
                op = bigp.tile([128, BLK], f32, tag="big")
                nc.tensor.matmul(op, at0[:, st * 128:(st + 1) * 128],
                                 ow_sb[:, hc * BLK:(hc + 1) * BLK],
                                 start=True, stop=False)
                nc.tensor.matmul(op, at1[:, st * 128:(st + 1) * 128],
                                 ow_sb[:, HID + hc * BLK:HID + (hc + 1) * BLK],
                                 start=False, stop=True)
                eng = (st * 5 + hc) % 2
                dst = ob[:, hc * BLK:(hc + 1) * BLK]
                if eng == 0:
                    nc.scalar.activation(dst, op, AF.Copy, bias=0.0, scale=1.0)
                else:
                    nc.vector.tensor_copy(dst, op)
            nc.sync.dma_start(
                out=outp[u * BLK + st * 128:u * BLK + (st + 1) * 128, :],
                in_=ob)

    prefetch(0)
    for t in range(NBLK):
        prefetch(t + 1)
        qTt = qpool.tile([128, 2 * BLK], bf16, tag="qT")
        qk_head(t, 0, 0, qTt)
        at_tiles[("q", t)] = qTt
        v_proj(t)
        kTt = kpool.tile([128, 2 * BLK], bf16, tag="kT")
        qk_head(t, 2, 2, kTt)
        k_tiles[t] = kTt
        h_tiles.pop(t)
        cs_tiles.pop(t)
        if t > 0:
            o_proj(t - 1)
        attention(t)
    o_proj(NBLK - 1)


def _build():
    nc = bacc.Bacc("TRN2", target_bir_lowering=False, debug=False,
                   num_devices=NCORES)
    hT = nc.dram_tensor("hT", [128, KT * S], bf16, kind="ExternalInput").ap()
    w = nc.dram_tensor("w", [128, KT * 768], bf16, kind="ExternalInput").ap()
    ow = nc.dram_tensor("ow", [128, 2 * HID], bf16, kind="ExternalInput").ap()
    cs = nc.dram_tensor("cs", [128, NBLK * 2 * BLK], bf16,
                        kind="ExternalInput").ap()
    msk = nc.dram_tensor("msk", [128, 256], bf16, kind="ExternalInput").ap()
    nw = nc.dram_tensor("nw", [128, 4], f32, kind="ExternalInput").ap()
    on = nc.dram_tensor("on", [128, 128], bf16, kind="ExternalInput").ap()
    outp = nc.dram_tensor("outp", [S, HID], bf16, kind="ExternalOutput").ap()
    with tile.TileContext(nc) as tc, ExitStack() as ctx:
        with nc.allow_low_precision(reason="bf16 matmul pipeline"):
            _body(ctx, tc, hT, w, ow, cs, msk, nw, on, outp)
    nc.compile()
    return nc


def _get_nc():
    global _NC
    if _NC is None:
        _NC = _build()
    return _NC


def kernel(positions, hidden_states, qkv_w, o_w, q_norm_w, k_norm_w):
    global _last_results
    _install_ntff_shim()

    positions = np.asarray(positions)
    hidden_states = np.asarray(hidden_states, dtype=np.float32)
    qkv_w = np.asarray(qkv_w, dtype=np.float32)
    o_w = np.asarray(o_w, dtype=np.float32)
    q_norm_w = np.asarray(q_norm_w, dtype=np.float32)
    k_norm_w = np.asarray(k_norm_w, dtype=np.float32)
    assert np.array_equal(positions.astype(np.int64), np.arange(S)), \
        "kernel assumes contiguous arange positions (banded sliding window)"

    hT0 = hidden_states.T  # [HID, S]
    hT = np.ascontiguousarray(
        hT0.reshape(KT, 128, NBLK, BLK).transpose(1, 2, 0, 3)
        .reshape(128, KT * S)).astype(bfnp)

    inv_freq = 1.0 / (ROPE_BASE ** (np.arange(0, HD, 2, dtype=np.float32) / HD))
    freqs = positions.astype(np.float32)[:, None] * inv_freq[None, :]  # [S,128]
    cos_t = np.cos(freqs).T.astype(np.float32)
    sin_t = np.sin(freqs).T.astype(np.float32)
    csb = np.stack([cos_t.reshape(128, NBLK, BLK),
                    sin_t.reshape(128, NBLK, BLK)], axis=2)
    cs = np.ascontiguousarray(
        csb.reshape(128, NBLK * 2 * BLK)).astype(bfnp)

    kl = np.arange(128)[:, None]
    ql = np.arange(128)[None, :]
    edge = (kl > ql).astype(np.float32)
    diag = (kl <= ql).astype(np.float32)
    msk = np.concatenate([edge, diag], axis=1).astype(bfnp)  # [128, 256]

    nwq = 1.0 + q_norm_w
    nwk = 1.0 + k_norm_w
    nw = np.stack([nwq[:128], nwq[128:], nwk[:128], nwk[128:]], axis=1)
    nw = np.ascontiguousarray(nw.astype(np.float32))  # [128, 4]

    on = np.ones((128, 128), bfnp)

    in_maps = []
    for c in range(NCORES):
        g = c // 2
        wq = qkv_w[:, c * HD:(c + 1) * HD]
        wk = qkv_w[:, NH * HD + g * HD:NH * HD + (g + 1) * HD]
        wv = qkv_w[:, (NH + NKV) * HD + g * HD:(NH + NKV) * HD + (g + 1) * HD]
        wslice = np.concatenate([wq, wk, wv], axis=1).astype(np.float32)
        wslice = np.ascontiguousarray(
            wslice.reshape(KT, 128, 768).transpose(1, 0, 2)
            .reshape(128, KT * 768)).astype(bfnp)
        owslice = o_w[c * HD:(c + 1) * HD, :].astype(np.float32)
        owslice = np.ascontiguousarray(
            owslice.reshape(2, 128, HID).transpose(1, 0, 2)
            .reshape(128, 2 * HID)).astype(bfnp)
        in_maps.append({
            "hT": hT, "w": wslice, "ow": owslice, "cs": cs, "msk": msk,
            "nw": nw, "on": on,
        })

    nc = _get_nc()
    res = run_bass_kernel_spmd(nc, in_maps, list(range(NCORES)))
    _last_results = res

    out = res.results[0]["outp"].astype(np.float32)
    for c in range(1, NCORES):
        out += res.results[c]["outp"].astype(np.float32)
    return out
